# revision 18
# baseline (speedup 1.0000x reference)
"""Fused single-head attention (QKV proj + softmax*scale + AV) on 8 trn2 cores.

Reference computation (fp32):
    qkv = x @ W.T            x:[4,4096,768]  W:[192,768]
    q,k,v = split(qkv, 64)
    A = q @ k.T              (no pre-softmax scale)
    out = softmax(A) / 8 @ v

Sharding: core c handles batch b=c//2, query half qh=c%2 (2048 queries),
full 4096 keys of that batch. SPMD-uniform program: the host rolls the
key/value columns of x^T by qh*2048 so every core's own queries are
always columns 0:2048 (softmax is permutation-invariant over keys).

Device dataflow (per core), matmuls in fp32r (1 cyc/col) or bf16:
    xt  [768, 4096] = x[b].T rolled
    wt  [768, 192]  = W[perm].T, perm = [K rows | V rows | Q rows]
    projection -> [K^T|V^T] (M=128) and Q^T (M=64) per 512-col block
    K^T stored pair-interleaved [128, 2048]: even k-tiles in rows 0:64,
    odd in rows 64:128; Q^T duplicated into both row halves. Scores for
    a k-tile pair run as two CONCURRENT row-group matmuls (contraction
    dh=64 needs only half the PE rows each).
    P^T = exp(A^T - 40) in bf16 (no row max needed: |A| <= ~77)
    out^T[dh,q] (+rowsum in row 64) accumulated in PSUM over k-tiles:
    lhsT = V_aug [128, 65] (col 64 = ones), rhs = P^T chunks.
    Finalize: out = PE-transpose(out^T) / (8 * rowsum).

Scheduling: the emission is software-pipelined -- AV of pair j is
emitted after the scores of pair j+1, so the PE never sits directly
behind the ACT exp; far-half projection and h0-finalize are interleaved
into the pair stream as PE filler.
"""

import sys

import numpy as np

for _p in ("/opt/trn_rl_repo",):
    if _p not in sys.path:
        sys.path.insert(0, _p)

import concourse.mybir as mybir  # noqa: E402
import concourse.tile as tile  # noqa: E402
from concourse import bacc  # noqa: E402
from concourse.bass_utils import run_bass_kernel_spmd  # noqa: E402
from concourse.masks import make_identity  # noqa: E402

B, S, D, DH = 4, 4096, 768, 64
QN = S // 2          # queries per core
NSB = 8              # 512-wide super-blocks of s
NKT = 32             # 128-wide key tiles
NPAIR = NKT // 2
HALF = 1024          # q-chunk for the main loop
EXP_BIAS = -40.0     # global score offset (softmax-invariant), fp32 headroom

F32 = mybir.dt.float32
F32R = mybir.dt.float32r
BF16 = mybir.dt.bfloat16

_NC_CACHE = None
LAST_RESULTS = None


def _build():
    nc = bacc.Bacc(num_devices=8)
    xt_d = nc.dram_tensor("xt", [D, S], F32R, kind="ExternalInput")
    wt_d = nc.dram_tensor("wt", [128, 6, 3 * DH], F32R, kind="ExternalInput")
    out_d = nc.dram_tensor("out", [QN, DH], F32, kind="ExternalOutput")

    with tile.TileContext(nc) as tc:
        with (
            tc.tile_pool(name="big", bufs=1) as big,
            tc.tile_pool(name="psmm", bufs=3, space="PSUM") as psmm,
            tc.tile_pool(name="psacc", bufs=1, space="PSUM") as psacc,
            tc.tile_pool(name="pt", bufs=4) as ptp,
            tc.tile_pool(name="small", bufs=4) as small,
        ):
            xt_tiles = []
            for _sb in range(NSB):
                _xt = big.tile([128, 6, 512], F32R, tag=f"xt{_sb}")
                xt_tiles.append(_xt)
            wt_sb = big.tile([128, 6, 3 * DH], F32R)
            ktp = big.tile([128, NPAIR * 128], F32R)  # pair-interleaved K^T
            qt_sb = big.tile([128, QN], F32R)         # Q^T duplicated rows
            vt_sb = big.tile([64, S], BF16)
            v_sb = big.tile([128, NKT, 80], BF16)     # [...,0:64]=V, 64=ones
            acc_sb = big.tile([65, QN], F32)
            osb = big.tile([128, 16, DH], F32)
            ident = big.tile([128, 128], F32)
            identb = big.tile([128, 128], BF16)

            make_identity(nc, ident[:])
            make_identity(nc, identb[:])
            nc.vector.memset(v_sb[:, :, 64:65], 1.0)
            ebias = big.tile([128, 1], F32)
            nc.vector.memset(ebias[:], EXP_BIAS)

            nc.sync.dma_start(out=wt_sb[:], in_=wt_d[:])

            def emit_proj(sb):
                sl = slice(sb * 512, (sb + 1) * 512)
                xt_t = xt_tiles[sb]
                dma_eng = nc.sync if sb % 2 == 0 else nc.scalar
                dma_eng.dma_start(
                    out=xt_t[:],
                    in_=xt_d[:, sl].rearrange("(k p) s -> p k s", p=128),
                )
                kv_ps = psmm.tile([128, HALF], F32, tag="mm")
                for k in range(6):
                    nc.tensor.matmul(
                        kv_ps[:, 0:512],
                        wt_sb[:, k, 0:128],
                        xt_t[:, k, :],
                        start=(k == 0),
                        stop=(k == 5),
                    )
                # K^T into pair-interleaved layout: tile t = 4*sb+i
                for i in range(4):
                    t = 4 * sb + i
                    rh = 64 * (t % 2)
                    nc.vector.tensor_copy(
                        ktp[rh:rh + 64, (t // 2) * 128:(t // 2 + 1) * 128],
                        kv_ps[0:64, i * 128:(i + 1) * 128],
                    )
                nc.vector.tensor_copy(vt_sb[:, sl], kv_ps[64:128, 0:512])
                if sb < 4:
                    q_ps = psmm.tile([64, HALF], F32, tag="mm")
                    for k in range(6):
                        nc.tensor.matmul(
                            q_ps[:, 0:512],
                            wt_sb[:, k, 128:192],
                            xt_t[:, k, :],
                            start=(k == 0),
                            stop=(k == 5),
                        )
                    nc.vector.tensor_copy(qt_sb[0:64, sl], q_ps[:, 0:512])
                    nc.vector.tensor_copy(qt_sb[64:128, sl], q_ps[:, 0:512])
                # V natural tiles via PE transpose (bf16), 4 k-tiles per sb.
                # Target the unused second bank of kv_ps (bf16 view) so the
                # transposes never contend for a fresh PSUM slot.
                kv16 = kv_ps[:].bitcast(BF16)
                for t4 in range(4):
                    t = sb * 4 + t4
                    tsl = slice(1024 + 64 * t4, 1024 + 64 * (t4 + 1))
                    nc.tensor.transpose(
                        kv16[:, tsl],
                        vt_sb[:, t * 128:(t + 1) * 128],
                        identb[0:64, 0:64],
                    )
                    nc.vector.tensor_copy(v_sb[:, t, 0:64], kv16[:, tsl])

            ats = {}
            pts = {}
            accs = {}

            def emit_scores(h, j):
                at_e = psmm.tile([128, HALF], F32, tag="mm")
                at_o = psmm.tile([128, HALF], F32, tag="mm")
                for g in range(2):
                    gsl = slice(h * HALF + g * 512, h * HALF + g * 512 + 512)
                    osl = slice(g * 512, (g + 1) * 512)
                    nc.tensor.matmul(
                        at_e[:, osl],
                        ktp[0:64, j * 128:(j + 1) * 128],
                        qt_sb[0:64, gsl],
                        start=True, stop=True,
                    )
                    nc.tensor.matmul(
                        at_o[:, osl],
                        ktp[64:128, j * 128:(j + 1) * 128],
                        qt_sb[64:128, gsl],
                        start=True, stop=True,
                    )
                pt_e = ptp.tile([128, HALF], BF16, tag="pt")
                pt_o = ptp.tile([128, HALF], BF16, tag="pt")
                nc.scalar.activation(
                    out=pt_e[:], in_=at_e[:],
                    func=mybir.ActivationFunctionType.Exp, bias=ebias[:],
                )
                nc.scalar.activation(
                    out=pt_o[:], in_=at_o[:],
                    func=mybir.ActivationFunctionType.Exp, bias=ebias[:],
                )
                pts[(h, j)] = (pt_e, pt_o)

            def emit_av(h, j):
                pt_e, pt_o = pts.pop((h, j))
                acc = accs[h]
                for tt, pt in ((2 * j, pt_e), (2 * j + 1, pt_o)):
                    for g in range(2):
                        osl = slice(g * 512, (g + 1) * 512)
                        nc.tensor.matmul(
                            acc[:, osl],
                            v_sb[:, tt, 0:65],
                            pt[:, osl],
                            start=(j == 0 and tt == 2 * j),
                            stop=(j == NPAIR - 1 and tt == 2 * j + 1),
                            skip_group_check=True,
                        )

            def emit_fin(gblk):
                ot = psmm.tile([128, HALF], F32, tag="mm")
                nc.tensor.transpose(
                    ot[:, 0:65],
                    acc_sb[:, gblk * 128:(gblk + 1) * 128],
                    ident[0:65, 0:65],
                )
                r = small.tile([128, 1], F32, tag="r")
                nc.vector.tensor_scalar_mul(r[:], ot[:, 64:65], 8.0)
                nc.vector.reciprocal(r[:], r[:])
                nc.vector.tensor_scalar_mul(osb[:, gblk, :], ot[:, 0:64], r[:])

            # ---- emission schedule (software-pipelined) ----
            # h0 scores use q cols 0:1024 (projected from sb0+sb1), and pair
            # j's k-tiles come from sb j//2 -- so the main loop can start as
            # soon as two super-blocks have landed. Remaining projection is
            # interleaved one super-block every other pair, pacing the DMA.
            pairs = [(0, j) for j in range(NPAIR)] + [(1, j) for j in range(NPAIR)]
            acc_h0 = psacc.tile([65, HALF], F32, tag="acc")
            accs[0] = acc_h0

            proj_at = {0: 0, 1: 2, 2: 3, 4: 4, 6: 5, 8: 6, 10: 7}
            emitted_proj = 0
            for i, (h, j) in enumerate(pairs):
                if i in proj_at:
                    emit_proj(proj_at[i])
                    emitted_proj += 1
                    if i == 0:
                        emit_proj(1)
                        emitted_proj += 1
                emit_scores(h, j)
                if i > 0:
                    ph, pj = pairs[i - 1]
                    emit_av(ph, pj)
                    if (ph, pj) == (0, NPAIR - 1):
                        # h0 accumulation complete: stage it and swap acc
                        nc.vector.tensor_copy(acc_sb[:, 0:512], accs[0][:, 0:512])
                        nc.vector.tensor_copy(acc_sb[:, 512:HALF], accs[0][:, 512:HALF])
                        acc_h1 = psacc.tile([65, HALF], F32, tag="acc")
                        accs[1] = acc_h1
                # h0 finalize (blocks 0..7) interleaved into h1 pair stream
                if h == 1 and j in (4, 8):
                    for blk in range(j - 4, j):
                        emit_fin(blk)
            assert emitted_proj == NSB
            emit_av(*pairs[-1])
            nc.vector.tensor_copy(acc_sb[:, HALF:HALF + 512], accs[1][:, 0:512])
            for gblk in range(8, 12):
                emit_fin(gblk)
            nc.vector.tensor_copy(acc_sb[:, HALF + 512:2 * HALF],
                                  accs[1][:, 512:HALF])
            for gblk in range(12, 16):
                emit_fin(gblk)

            nc.sync.dma_start(
                out=out_d[:].rearrange("(t p) d -> p t d", p=128), in_=osb[:]
            )

    nc.finalize()
    return nc


def _get_nc():
    global _NC_CACHE
    if _NC_CACHE is None:
        _NC_CACHE = _build()
    return _NC_CACHE


def kernel(x, W, _trace=False):
    global LAST_RESULTS
    x = np.ascontiguousarray(np.asarray(x), dtype=np.float32)
    W = np.ascontiguousarray(np.asarray(W), dtype=np.float32)
    assert x.shape == (B, S, D) and W.shape == (3 * DH, D)

    # wt columns: [K | V | Q] so proj M-tile0 = [K^T|V^T], M-tile1 = Q^T
    wtf = np.concatenate([W[DH:2 * DH], W[2 * DH:], W[:DH]], axis=0).T
    wt = np.ascontiguousarray(wtf.reshape(6, 128, 3 * DH).transpose(1, 0, 2))

    in_maps = []
    for c in range(8):
        b, qh = divmod(c, 2)
        xtb = x[b].T  # [768, 4096]
        if qh:
            xtc = np.ascontiguousarray(
                np.concatenate([xtb[:, QN:], xtb[:, :QN]], axis=1)
            )
        else:
            xtc = np.ascontiguousarray(xtb)
        in_maps.append({"xt": xtc, "wt": wt})

    nc = _get_nc()
    res = run_bass_kernel_spmd(nc, in_maps, list(range(8)), trace=_trace)
    LAST_RESULTS = res

    out = np.empty((B, S, DH), np.float32)
    for c in range(8):
        b, qh = divmod(c, 2)
        out[b, qh * QN:(qh + 1) * QN] = res.results[c]["out"]
    return out


# revision 19
# speedup vs baseline: 1.1835x; 1.1835x over previous
"""Fused single-head attention (QKV proj + softmax*scale + AV) on 8 trn2 cores.

Reference computation (fp32):
    qkv = x @ W.T            x:[4,4096,768]  W:[192,768]
    q,k,v = split(qkv, 64)
    A = q @ k.T              (no pre-softmax scale)
    out = softmax(A) / 8 @ v

Sharding: core c handles batch b=c//2, query half qh=c%2 (2048 queries),
full 4096 keys of that batch. SPMD-uniform program: the host rolls the
key/value columns of x^T by qh*2048 so every core's own queries are
always columns 0:2048 (softmax is permutation-invariant over keys).

Device dataflow (per core), matmuls in fp32r (1 cyc/col) or bf16:
    xt  [768, 4096] = x[b].T rolled
    wt  [768, 192]  = W[perm].T, perm = [K rows | V rows | Q rows]
    projection -> [K^T|V^T] (M=128) and Q^T (M=64) per 512-col block
    K^T stored pair-interleaved [128, 2048]: even k-tiles in rows 0:64,
    odd in rows 64:128; Q^T duplicated into both row halves. Scores for
    a k-tile pair run as two CONCURRENT row-group matmuls (contraction
    dh=64 needs only half the PE rows each).
    P^T = exp(A^T - 40) in bf16 (no row max needed: |A| <= ~77)
    out^T[dh,q] (+rowsum in row 64) accumulated in PSUM over k-tiles:
    lhsT = V_aug [128, 65] (col 64 = ones), rhs = P^T chunks.
    Finalize: out = PE-transpose(out^T) / (8 * rowsum).

Scheduling: the emission is software-pipelined -- AV of pair j is
emitted after the scores of pair j+1, so the PE never sits directly
behind the ACT exp; far-half projection and h0-finalize are interleaved
into the pair stream as PE filler.
"""

import sys

import numpy as np

for _p in ("/opt/trn_rl_repo",):
    if _p not in sys.path:
        sys.path.insert(0, _p)

import concourse.mybir as mybir  # noqa: E402
import concourse.tile as tile  # noqa: E402
from concourse import bacc  # noqa: E402
from concourse.bass_utils import run_bass_kernel_spmd  # noqa: E402
from concourse.masks import make_identity  # noqa: E402

B, S, D, DH = 4, 4096, 768, 64
QN = S // 2          # queries per core
NSB = 8              # 512-wide super-blocks of s
NKT = 32             # 128-wide key tiles
NPAIR = NKT // 2
HALF = 1024          # q-chunk for the main loop
EXP_BIAS = -40.0     # global score offset (softmax-invariant), fp32 headroom

F32 = mybir.dt.float32
F32R = mybir.dt.float32r
BF16 = mybir.dt.bfloat16

_NC_CACHE = None
LAST_RESULTS = None


def _build():
    nc = bacc.Bacc(num_devices=8)
    xt_d = nc.dram_tensor("xt", [D, S], F32R, kind="ExternalInput")
    wt_d = nc.dram_tensor("wt", [128, 6, 3 * DH], F32R, kind="ExternalInput")
    out_d = nc.dram_tensor("out", [QN, DH], F32, kind="ExternalOutput")

    with tile.TileContext(nc) as tc:
        with (
            tc.tile_pool(name="big", bufs=1) as big,
            tc.tile_pool(name="psmm", bufs=3, space="PSUM") as psmm,
            tc.tile_pool(name="psacc", bufs=1, space="PSUM") as psacc,
            tc.tile_pool(name="pt", bufs=4) as ptp,
            tc.tile_pool(name="small", bufs=4) as small,
        ):
            xt_tiles = []
            for _sb in range(NSB):
                _xt = big.tile([128, 6, 512], F32R, tag=f"xt{_sb}")
                xt_tiles.append(_xt)
            wt_sb = big.tile([128, 6, 3 * DH], F32R)
            ktp = big.tile([128, NPAIR * 128], F32R)  # pair-interleaved K^T
            qt_sb = big.tile([128, QN], F32R)         # Q^T duplicated rows
            vt_sb = big.tile([64, S], BF16)
            v_sb = big.tile([128, NKT, 80], BF16)     # [...,0:64]=V, 64=ones
            acc_sb = big.tile([65, QN], F32)
            osb = big.tile([128, 16, DH], F32)
            ident = big.tile([128, 128], F32)
            identb = big.tile([128, 128], BF16)

            make_identity(nc, ident[:])
            make_identity(nc, identb[:])
            nc.vector.memset(v_sb[:, :, 64:65], 1.0)
            ebias = big.tile([128, 1], F32)
            nc.vector.memset(ebias[:], EXP_BIAS)

            nc.sync.dma_start(out=wt_sb[:], in_=wt_d[:])

            def emit_proj(sb):
                sl = slice(sb * 512, (sb + 1) * 512)
                xt_t = xt_tiles[sb]
                nc.sync.dma_start(
                    out=xt_t[:],
                    in_=xt_d[:, sl].rearrange("(k p) s -> p k s", p=128),
                )
                kv_ps = psmm.tile([128, HALF], F32, tag="mm")
                for k in range(6):
                    nc.tensor.matmul(
                        kv_ps[:, 0:512],
                        wt_sb[:, k, 0:128],
                        xt_t[:, k, :],
                        start=(k == 0),
                        stop=(k == 5),
                    )
                # K^T into pair-interleaved layout: tile t = 4*sb+i
                for i in range(4):
                    t = 4 * sb + i
                    rh = 64 * (t % 2)
                    nc.vector.tensor_copy(
                        ktp[rh:rh + 64, (t // 2) * 128:(t // 2 + 1) * 128],
                        kv_ps[0:64, i * 128:(i + 1) * 128],
                    )
                nc.vector.tensor_copy(vt_sb[:, sl], kv_ps[64:128, 0:512])
                if sb < 4:
                    q_ps = psmm.tile([64, HALF], F32, tag="mm")
                    for k in range(6):
                        nc.tensor.matmul(
                            q_ps[:, 0:512],
                            wt_sb[:, k, 128:192],
                            xt_t[:, k, :],
                            start=(k == 0),
                            stop=(k == 5),
                        )
                    nc.vector.tensor_copy(qt_sb[0:64, sl], q_ps[:, 0:512])
                    nc.vector.tensor_copy(qt_sb[64:128, sl], q_ps[:, 0:512])
                # V natural tiles via PE transpose (bf16), 4 k-tiles per sb.
                # Target the unused second bank of kv_ps (bf16 view) so the
                # transposes never contend for a fresh PSUM slot.
                kv16 = kv_ps[:].bitcast(BF16)
                for t4 in range(4):
                    t = sb * 4 + t4
                    tsl = slice(1024 + 64 * t4, 1024 + 64 * (t4 + 1))
                    nc.tensor.transpose(
                        kv16[:, tsl],
                        vt_sb[:, t * 128:(t + 1) * 128],
                        identb[0:64, 0:64],
                    )
                    nc.vector.tensor_copy(v_sb[:, t, 0:64], kv16[:, tsl])

            ats = {}
            pts = {}
            accs = {}

            def emit_scores(h, j):
                at_e = psmm.tile([128, HALF], F32, tag="mm")
                at_o = psmm.tile([128, HALF], F32, tag="mm")
                for g in range(2):
                    gsl = slice(h * HALF + g * 512, h * HALF + g * 512 + 512)
                    osl = slice(g * 512, (g + 1) * 512)
                    nc.tensor.matmul(
                        at_e[:, osl],
                        ktp[0:64, j * 128:(j + 1) * 128],
                        qt_sb[0:64, gsl],
                        start=True, stop=True,
                    )
                    nc.tensor.matmul(
                        at_o[:, osl],
                        ktp[64:128, j * 128:(j + 1) * 128],
                        qt_sb[64:128, gsl],
                        start=True, stop=True,
                    )
                pt_e = ptp.tile([128, HALF], BF16, tag="pt")
                pt_o = ptp.tile([128, HALF], BF16, tag="pt")
                nc.scalar.activation(
                    out=pt_e[:], in_=at_e[:],
                    func=mybir.ActivationFunctionType.Exp, bias=ebias[:],
                )
                nc.scalar.activation(
                    out=pt_o[:], in_=at_o[:],
                    func=mybir.ActivationFunctionType.Exp, bias=ebias[:],
                )
                pts[(h, j)] = (pt_e, pt_o)

            def emit_av(h, j):
                pt_e, pt_o = pts.pop((h, j))
                acc = accs[h]
                for tt, pt in ((2 * j, pt_e), (2 * j + 1, pt_o)):
                    for g in range(2):
                        osl = slice(g * 512, (g + 1) * 512)
                        nc.tensor.matmul(
                            acc[:, osl],
                            v_sb[:, tt, 0:65],
                            pt[:, osl],
                            start=(j == 0 and tt == 2 * j),
                            stop=(j == NPAIR - 1 and tt == 2 * j + 1),
                            skip_group_check=True,
                        )

            def emit_fin(gblk):
                ot = psmm.tile([128, HALF], F32, tag="mm")
                nc.tensor.transpose(
                    ot[:, 0:65],
                    acc_sb[:, gblk * 128:(gblk + 1) * 128],
                    ident[0:65, 0:65],
                )
                r = small.tile([128, 1], F32, tag="r")
                nc.vector.tensor_scalar_mul(r[:], ot[:, 64:65], 8.0)
                nc.vector.reciprocal(r[:], r[:])
                nc.vector.tensor_scalar_mul(osb[:, gblk, :], ot[:, 0:64], r[:])

            # ---- emission schedule (software-pipelined) ----
            # h0 scores use q cols 0:1024 (projected from sb0+sb1), and pair
            # j's k-tiles come from sb j//2 -- so the main loop can start as
            # soon as two super-blocks have landed. Remaining projection is
            # interleaved one super-block every other pair, pacing the DMA.
            pairs = [(0, j) for j in range(NPAIR)] + [(1, j) for j in range(NPAIR)]
            acc_h0 = psacc.tile([65, HALF], F32, tag="acc")
            accs[0] = acc_h0

            proj_at = {0: 0, 1: 2, 2: 3, 4: 4, 6: 5, 8: 6, 10: 7}
            emitted_proj = 0
            for i, (h, j) in enumerate(pairs):
                if i in proj_at:
                    emit_proj(proj_at[i])
                    emitted_proj += 1
                    if i == 0:
                        emit_proj(1)
                        emitted_proj += 1
                emit_scores(h, j)
                if i > 0:
                    ph, pj = pairs[i - 1]
                    emit_av(ph, pj)
                    if (ph, pj) == (0, NPAIR - 1):
                        # h0 accumulation complete: stage it and swap acc
                        nc.vector.tensor_copy(acc_sb[:, 0:512], accs[0][:, 0:512])
                        nc.vector.tensor_copy(acc_sb[:, 512:HALF], accs[0][:, 512:HALF])
                        acc_h1 = psacc.tile([65, HALF], F32, tag="acc")
                        accs[1] = acc_h1
                # h0 finalize (blocks 0..7) interleaved into h1 pair stream
                if h == 1 and j in (4, 8):
                    for blk in range(j - 4, j):
                        emit_fin(blk)
            assert emitted_proj == NSB
            emit_av(*pairs[-1])
            nc.vector.tensor_copy(acc_sb[:, HALF:HALF + 512], accs[1][:, 0:512])
            for gblk in range(8, 12):
                emit_fin(gblk)
            nc.vector.tensor_copy(acc_sb[:, HALF + 512:2 * HALF],
                                  accs[1][:, 512:HALF])
            for gblk in range(12, 16):
                emit_fin(gblk)

            nc.sync.dma_start(
                out=out_d[:].rearrange("(t p) d -> p t d", p=128), in_=osb[:]
            )

    nc.finalize()
    return nc


def _get_nc():
    global _NC_CACHE
    if _NC_CACHE is None:
        _NC_CACHE = _build()
    return _NC_CACHE


def kernel(x, W, _trace=False):
    global LAST_RESULTS
    x = np.ascontiguousarray(np.asarray(x), dtype=np.float32)
    W = np.ascontiguousarray(np.asarray(W), dtype=np.float32)
    assert x.shape == (B, S, D) and W.shape == (3 * DH, D)

    # wt columns: [K | V | Q] so proj M-tile0 = [K^T|V^T], M-tile1 = Q^T
    wtf = np.concatenate([W[DH:2 * DH], W[2 * DH:], W[:DH]], axis=0).T
    wt = np.ascontiguousarray(wtf.reshape(6, 128, 3 * DH).transpose(1, 0, 2))

    in_maps = []
    for c in range(8):
        b, qh = divmod(c, 2)
        xtb = x[b].T  # [768, 4096]
        if qh:
            xtc = np.ascontiguousarray(
                np.concatenate([xtb[:, QN:], xtb[:, :QN]], axis=1)
            )
        else:
            xtc = np.ascontiguousarray(xtb)
        in_maps.append({"xt": xtc, "wt": wt})

    nc = _get_nc()
    res = run_bass_kernel_spmd(nc, in_maps, list(range(8)), trace=_trace)
    LAST_RESULTS = res

    out = np.empty((B, S, DH), np.float32)
    for c in range(8):
        b, qh = divmod(c, 2)
        out[b, qh * QN:(qh + 1) * QN] = res.results[c]["out"]
    return out


# revision 20
# speedup vs baseline: 1.2070x; 1.0199x over previous
"""Fused single-head attention (QKV proj + softmax*scale + AV) on 8 trn2 cores.

Reference computation (fp32):
    qkv = x @ W.T            x:[4,4096,768]  W:[192,768]
    q,k,v = split(qkv, 64)
    A = q @ k.T              (no pre-softmax scale)
    out = softmax(A) / 8 @ v

Sharding: core c handles batch b=c//2, query half qh=c%2 (2048 queries),
full 4096 keys of that batch. SPMD-uniform program: the host rolls the
key/value columns of x^T by qh*2048 so every core's own queries are
always columns 0:2048 (softmax is permutation-invariant over keys).

Device dataflow (per core), matmuls in fp32r (1 cyc/col) or bf16:
    xt  [768, 4096] = x[b].T rolled
    wt  [768, 192]  = W[perm].T, perm = [K rows | V rows | Q rows]
    projection -> [K^T|V^T] (M=128) and Q^T (M=64) per 512-col block
    K^T stored pair-interleaved [128, 2048]: even k-tiles in rows 0:64,
    odd in rows 64:128; Q^T duplicated into both row halves. Scores for
    a k-tile pair run as two CONCURRENT row-group matmuls (contraction
    dh=64 needs only half the PE rows each).
    P^T = exp(A^T - 40) in bf16 (no row max needed: |A| <= ~77)
    out^T[dh,q] (+rowsum in row 64) accumulated in PSUM over k-tiles:
    lhsT = V_aug [128, 65] (col 64 = ones), rhs = P^T chunks.
    Finalize: out = PE-transpose(out^T) / (8 * rowsum).

Scheduling: the emission is software-pipelined -- AV of pair j is
emitted after the scores of pair j+1, so the PE never sits directly
behind the ACT exp; far-half projection and h0-finalize are interleaved
into the pair stream as PE filler.
"""

import sys

import numpy as np

for _p in ("/opt/trn_rl_repo",):
    if _p not in sys.path:
        sys.path.insert(0, _p)

import concourse.mybir as mybir  # noqa: E402
import concourse.tile as tile  # noqa: E402
from concourse import bacc  # noqa: E402
from concourse.bass_utils import run_bass_kernel_spmd  # noqa: E402
from concourse.masks import make_identity  # noqa: E402

B, S, D, DH = 4, 4096, 768, 64
QN = S // 2          # queries per core
NSB = 8              # 512-wide super-blocks of s
NKT = 32             # 128-wide key tiles
NPAIR = NKT // 2
HALF = 1024          # q-chunk for the main loop
EXP_BIAS = -40.0     # global score offset (softmax-invariant), fp32 headroom

F32 = mybir.dt.float32
F32R = mybir.dt.float32r
BF16 = mybir.dt.bfloat16

_NC_CACHE = None
LAST_RESULTS = None


def _build():
    nc = bacc.Bacc(num_devices=8)
    xt_d = nc.dram_tensor("xt", [D, S], F32R, kind="ExternalInput")
    wt_d = nc.dram_tensor("wt", [128, 6, 3 * DH], F32R, kind="ExternalInput")
    out_d = nc.dram_tensor("out", [QN, DH], F32, kind="ExternalOutput")

    with tile.TileContext(nc) as tc:
        with (
            tc.tile_pool(name="big", bufs=1) as big,
            tc.tile_pool(name="psmm", bufs=3, space="PSUM") as psmm,
            tc.tile_pool(name="psacc", bufs=1, space="PSUM") as psacc,
            tc.tile_pool(name="pt", bufs=4) as ptp,
            tc.tile_pool(name="small", bufs=4) as small,
        ):
            xt_tiles = []
            for _sb in range(NSB):
                _xt = big.tile([128, 6, 512], F32R, tag=f"xt{_sb}")
                xt_tiles.append(_xt)
            wt_sb = big.tile([128, 6, 3 * DH], F32R)
            ktp = big.tile([128, NPAIR * 128], F32R)  # pair-interleaved K^T
            qt_sb = big.tile([128, QN], F32R)         # Q^T duplicated rows
            vt_sb = big.tile([64, S], BF16)
            v_sb = big.tile([128, NKT, 80], BF16)     # [...,0:64]=V, 64=ones
            acc_sb = big.tile([65, QN], F32)
            osb = big.tile([128, 16, DH], F32)
            ident = big.tile([128, 128], F32)
            identb = big.tile([128, 128], BF16)

            make_identity(nc, ident[:])
            make_identity(nc, identb[:])
            nc.vector.memset(v_sb[:, :, 64:65], 1.0)
            ebias = big.tile([128, 1], F32)
            nc.vector.memset(ebias[:], EXP_BIAS)

            nc.sync.dma_start(out=wt_sb[:], in_=wt_d[:])

            def emit_proj(sb):
                sl = slice(sb * 512, (sb + 1) * 512)
                xt_t = xt_tiles[sb]
                for k in range(6):
                    nc.sync.dma_start(
                        out=xt_t[:, k, :],
                        in_=xt_d[k * 128:(k + 1) * 128, sl],
                    )
                kv_ps = psmm.tile([128, HALF], F32, tag="mm")
                for k in range(6):
                    nc.tensor.matmul(
                        kv_ps[:, 0:512],
                        wt_sb[:, k, 0:128],
                        xt_t[:, k, :],
                        start=(k == 0),
                        stop=(k == 5),
                    )
                # K^T into pair-interleaved layout: tile t = 4*sb+i
                for i in range(4):
                    t = 4 * sb + i
                    rh = 64 * (t % 2)
                    nc.vector.tensor_copy(
                        ktp[rh:rh + 64, (t // 2) * 128:(t // 2 + 1) * 128],
                        kv_ps[0:64, i * 128:(i + 1) * 128],
                    )
                nc.vector.tensor_copy(vt_sb[:, sl], kv_ps[64:128, 0:512])
                if sb < 4:
                    q_ps = psmm.tile([64, HALF], F32, tag="mm")
                    for k in range(6):
                        nc.tensor.matmul(
                            q_ps[:, 0:512],
                            wt_sb[:, k, 128:192],
                            xt_t[:, k, :],
                            start=(k == 0),
                            stop=(k == 5),
                        )
                    nc.vector.tensor_copy(qt_sb[0:64, sl], q_ps[:, 0:512])
                    nc.vector.tensor_copy(qt_sb[64:128, sl], q_ps[:, 0:512])
                # V natural tiles via PE transpose (bf16), 4 k-tiles per sb.
                # Target the unused second bank of kv_ps (bf16 view) so the
                # transposes never contend for a fresh PSUM slot.
                kv16 = kv_ps[:].bitcast(BF16)
                for t4 in range(4):
                    t = sb * 4 + t4
                    tsl = slice(1024 + 64 * t4, 1024 + 64 * (t4 + 1))
                    nc.tensor.transpose(
                        kv16[:, tsl],
                        vt_sb[:, t * 128:(t + 1) * 128],
                        identb[0:64, 0:64],
                    )
                    nc.vector.tensor_copy(v_sb[:, t, 0:64], kv16[:, tsl])

            ats = {}
            pts = {}
            accs = {}

            def emit_scores(h, j):
                at_e = psmm.tile([128, HALF], F32, tag="mm")
                at_o = psmm.tile([128, HALF], F32, tag="mm")
                for g in range(2):
                    gsl = slice(h * HALF + g * 512, h * HALF + g * 512 + 512)
                    osl = slice(g * 512, (g + 1) * 512)
                    nc.tensor.matmul(
                        at_e[:, osl],
                        ktp[0:64, j * 128:(j + 1) * 128],
                        qt_sb[0:64, gsl],
                        start=True, stop=True,
                    )
                    nc.tensor.matmul(
                        at_o[:, osl],
                        ktp[64:128, j * 128:(j + 1) * 128],
                        qt_sb[64:128, gsl],
                        start=True, stop=True,
                    )
                pt_e = ptp.tile([128, HALF], BF16, tag="pt")
                pt_o = ptp.tile([128, HALF], BF16, tag="pt")
                nc.scalar.activation(
                    out=pt_e[:], in_=at_e[:],
                    func=mybir.ActivationFunctionType.Exp, bias=ebias[:],
                )
                nc.scalar.activation(
                    out=pt_o[:], in_=at_o[:],
                    func=mybir.ActivationFunctionType.Exp, bias=ebias[:],
                )
                pts[(h, j)] = (pt_e, pt_o)

            def emit_av(h, j):
                pt_e, pt_o = pts.pop((h, j))
                acc = accs[h]
                for tt, pt in ((2 * j, pt_e), (2 * j + 1, pt_o)):
                    for g in range(2):
                        osl = slice(g * 512, (g + 1) * 512)
                        nc.tensor.matmul(
                            acc[:, osl],
                            v_sb[:, tt, 0:65],
                            pt[:, osl],
                            start=(j == 0 and tt == 2 * j),
                            stop=(j == NPAIR - 1 and tt == 2 * j + 1),
                            skip_group_check=True,
                        )

            def emit_fin(gblk):
                ot = psmm.tile([128, HALF], F32, tag="mm")
                nc.tensor.transpose(
                    ot[:, 0:65],
                    acc_sb[:, gblk * 128:(gblk + 1) * 128],
                    ident[0:65, 0:65],
                )
                r = small.tile([128, 1], F32, tag="r")
                nc.vector.tensor_scalar_mul(r[:], ot[:, 64:65], 8.0)
                nc.vector.reciprocal(r[:], r[:])
                nc.vector.tensor_scalar_mul(osb[:, gblk, :], ot[:, 0:64], r[:])

            # ---- emission schedule (software-pipelined) ----
            # h0 scores use q cols 0:1024 (projected from sb0+sb1), and pair
            # j's k-tiles come from sb j//2 -- so the main loop can start as
            # soon as two super-blocks have landed. Remaining projection is
            # interleaved one super-block every other pair, pacing the DMA.
            pairs = [(0, j) for j in range(NPAIR)] + [(1, j) for j in range(NPAIR)]
            acc_h0 = psacc.tile([65, HALF], F32, tag="acc")
            accs[0] = acc_h0

            proj_at = {0: 0, 1: 2, 2: 3, 4: 4, 6: 5, 8: 6, 10: 7}
            emitted_proj = 0
            for i, (h, j) in enumerate(pairs):
                if i in proj_at:
                    emit_proj(proj_at[i])
                    emitted_proj += 1
                    if i == 0:
                        emit_proj(1)
                        emitted_proj += 1
                emit_scores(h, j)
                if i > 0:
                    ph, pj = pairs[i - 1]
                    emit_av(ph, pj)
                    if (ph, pj) == (0, NPAIR - 1):
                        # h0 accumulation complete: stage it and swap acc
                        nc.vector.tensor_copy(acc_sb[:, 0:512], accs[0][:, 0:512])
                        nc.vector.tensor_copy(acc_sb[:, 512:HALF], accs[0][:, 512:HALF])
                        acc_h1 = psacc.tile([65, HALF], F32, tag="acc")
                        accs[1] = acc_h1
                # h0 finalize (blocks 0..7) interleaved into h1 pair stream
                if h == 1 and j in (4, 8):
                    for blk in range(j - 4, j):
                        emit_fin(blk)
            assert emitted_proj == NSB
            emit_av(*pairs[-1])
            nc.vector.tensor_copy(acc_sb[:, HALF:HALF + 512], accs[1][:, 0:512])
            for gblk in range(8, 12):
                emit_fin(gblk)
            nc.vector.tensor_copy(acc_sb[:, HALF + 512:2 * HALF],
                                  accs[1][:, 512:HALF])
            for gblk in range(12, 16):
                emit_fin(gblk)

            nc.sync.dma_start(
                out=out_d[:].rearrange("(t p) d -> p t d", p=128), in_=osb[:]
            )

    nc.finalize()
    return nc


def _get_nc():
    global _NC_CACHE
    if _NC_CACHE is None:
        _NC_CACHE = _build()
    return _NC_CACHE


def kernel(x, W, _trace=False):
    global LAST_RESULTS
    x = np.ascontiguousarray(np.asarray(x), dtype=np.float32)
    W = np.ascontiguousarray(np.asarray(W), dtype=np.float32)
    assert x.shape == (B, S, D) and W.shape == (3 * DH, D)

    # wt columns: [K | V | Q] so proj M-tile0 = [K^T|V^T], M-tile1 = Q^T
    wtf = np.concatenate([W[DH:2 * DH], W[2 * DH:], W[:DH]], axis=0).T
    wt = np.ascontiguousarray(wtf.reshape(6, 128, 3 * DH).transpose(1, 0, 2))

    in_maps = []
    for c in range(8):
        b, qh = divmod(c, 2)
        xtb = x[b].T  # [768, 4096]
        if qh:
            xtc = np.ascontiguousarray(
                np.concatenate([xtb[:, QN:], xtb[:, :QN]], axis=1)
            )
        else:
            xtc = np.ascontiguousarray(xtb)
        in_maps.append({"xt": xtc, "wt": wt})

    nc = _get_nc()
    res = run_bass_kernel_spmd(nc, in_maps, list(range(8)), trace=_trace)
    LAST_RESULTS = res

    out = np.empty((B, S, DH), np.float32)
    for c in range(8):
        b, qh = divmod(c, 2)
        out[b, qh * QN:(qh + 1) * QN] = res.results[c]["out"]
    return out
